# revision 11
# baseline (speedup 1.0000x reference)
"""BiAttention Trainium2 Bass kernel.

Per-core (one batch per NeuronCore, batch=8 over 8 cores):
  att[i,j] = input_dot[i] + memory_dot[j] + (input*dot_scale) @ memory^T - NEG*(1-mask[j])
  weight_one = softmax_j(att);  output_one = weight_one @ memory
  weight_two = softmax_i(max_j att);  output_two = weight_two @ input
  out = concat([input, output_one, input*output_one, output_two*output_one], -1)

Implementation notes:
  - input_dot[i] is constant along j, so it cancels in softmax_j; only
    memory_dot + mask enter the attention bias (per-j "mvec").
  - Rows of `memory` (and the additive mask) are permuted host-side so that
    unmasked rows come first; masked rows never reach the device (sum over j is
    permutation invariant).  Only Lmp = ceil(count/128)*128 rows are computed.
  - Scores are built transposed (S^T[j,i]) so mvec is a per-partition ACT bias
    and exp(S^T + mvec - C) lands directly in the P^T layout that the second
    matmul (contraction over j) needs.  C = max(mvec)+4 is a safe global shift.
  - max_j att (needed for weight_two) is recovered as C + log(max_j expvals);
    the log never materializes: weight_two numerator uses maxP * exp(input_dot-K).
  - denominator sum_j comes for free from an appended ones-column in memory.
"""

import math
import numpy as np

import concourse.bass as bass
import concourse.mybir as mybir
import concourse.tile as tile
import concourse.bacc as bacc
from concourse import bass_isa
from concourse.bass_utils import run_bass_kernel_spmd
from concourse.masks import make_identity

F32 = mybir.dt.float32
BF16 = mybir.dt.bfloat16
AX = mybir.AxisListType
ALU = mybir.AluOpType
ACTF = mybir.ActivationFunctionType

N_CORES = 8
NEG = 1e30

_NC_CACHE: dict = {}
LAST_RESULTS = None  # BassKernelResults of the most recent run (for test harness)


def build_nc(Li: int, Lmp: int, d: int):
    """Build the single-core SPMD program.  Li, d fixed; Lmp = padded #unmasked."""
    assert Li % 128 == 0 and Lmp % 128 == 0 and d == 256
    NI = Li // 128
    NJ = Lmp // 128
    banks = [(s, min(512, Li - s)) for s in range(0, Li, 512)]

    nc = bacc.Bacc("TRN2", target_bir_lowering=False, debug=False,
                   num_devices=N_CORES)

    x_d = nc.dram_tensor("x", [Li, d], F32, kind="ExternalInput")
    m_d = nc.dram_tensor("m", [Lmp, d], F32, kind="ExternalInput")
    mp_d = nc.dram_tensor("mp", [128, NJ], F32, kind="ExternalInput")
    win_d = nc.dram_tensor("w_in", [d], F32, kind="ExternalInput")
    wmem_d = nc.dram_tensor("w_mem", [d], F32, kind="ExternalInput")
    dsc_d = nc.dram_tensor("dsc", [d], F32, kind="ExternalInput")
    out_d = nc.dram_tensor("out", [Li, 4 * d], F32, kind="ExternalOutput")

    with tile.TileContext(nc) as tc:
        with (
            tc.tile_pool(name="singles", bufs=1) as singles,
            tc.tile_pool(name="mload", bufs=3) as mload,
            tc.tile_pool(name="xsb", bufs=3) as xsb,
            tc.tile_pool(name="scr", bufs=3) as scr,
            tc.tile_pool(name="stg", bufs=3) as stgp,
            tc.tile_pool(name="ps", bufs=2, space="PSUM") as ps,
        ):
            # ---- constants / broadcasts ----
            win_b = singles.tile([128, d], F32, tag="win_b")
            wmem_b = singles.tile([128, d], F32, tag="wmem_b")
            dsc_b = singles.tile([128, d], F32, tag="dsc_b")
            nc.sync.dma_start(out=win_b, in_=win_d.ap().unsqueeze(0).partition_broadcast(128))
            nc.sync.dma_start(out=wmem_b, in_=wmem_d.ap().unsqueeze(0).partition_broadcast(128))
            nc.sync.dma_start(out=dsc_b, in_=dsc_d.ap().unsqueeze(0).partition_broadcast(128))

            ident = singles.tile([128, 128], BF16, tag="ident")
            make_identity(nc, ident)
            ident32 = singles.tile([128, 128], F32, tag="ident32")
            make_identity(nc, ident32)
            ones32 = singles.tile([128, 1], F32, tag="ones32")
            nc.vector.memset(ones32, 1.0)

            mp_sb = singles.tile([128, NJ], F32, tag="mp_sb")
            nc.sync.dma_start(out=mp_sb, in_=mp_d[:, :])

            # ---- resident big tiles ----
            x_all = singles.tile([128, NI * d], F32, tag="x_all")
            xb_all = singles.tile([128, NI * d], BF16, tag="xb_all")
            inputT = singles.tile([128, 2 * Li], BF16, tag="inputT")  # [d-half, i]
            memT = singles.tile([128, 2 * Lmp], BF16, tag="memT")     # [d-half, j]
            maug = singles.tile([128, NJ * (d + 1)], BF16, tag="maug")
            PT = singles.tile([128, NJ * Li], BF16, tag="PT")         # exp scores^T
            M1 = singles.tile([128, Li], BF16, tag="M1")              # running max of PT

            # ---- small stats ----
            idot = singles.tile([128, NI], F32, tag="idot")
            mvec = singles.tile([128, NJ], F32, tag="mvec")
            bias_sb = singles.tile([128, NJ], F32, tag="bias_sb")
            maxP = singles.tile([128, NI], F32, tag="maxP")
            cmax = singles.tile([128, 1], F32, tag="cmax")
            cm1 = singles.tile([1, 1], F32, tag="cm1")
            cm_all = singles.tile([128, 1], F32, tag="cm_all")
            k1 = singles.tile([128, 1], F32, tag="k1")
            k11 = singles.tile([1, 1], F32, tag="k11")
            k_all = singles.tile([128, 1], F32, tag="k_all")
            negk = singles.tile([128, 1], F32, tag="negk")
            e2 = singles.tile([128, NI], F32, tag="e2")
            u_t = singles.tile([128, NI], F32, tag="u_t")
            su1 = singles.tile([128, 1], F32, tag="su1")
            su11 = singles.tile([1, 1], F32, tag="su11")
            su_all = singles.tile([128, 1], F32, tag="su_all")
            rec2 = singles.tile([128, 1], F32, tag="rec2")
            wt2b = singles.tile([128, NI], BF16, tag="wt2b")
            o2_1 = singles.tile([1, d], F32, tag="o2_1")
            o2b = singles.tile([128, d], F32, tag="o2b")

            # ---- stage A: load input, input_dot, casts, transposes ----
            for ic in range(NI):
                x_sl = x_all[:, ic * d:(ic + 1) * d]
                nc.sync.dma_start(out=x_sl, in_=x_d[ic * 128:(ic + 1) * 128, :])
                sc = scr.tile([128, d], F32, tag="scr")
                nc.vector.scalar_tensor_tensor(
                    out=sc, in0=x_sl, scalar=0.0, in1=win_b,
                    op0=ALU.add, op1=ALU.mult, accum_out=idot[:, ic:ic + 1])
                nc.scalar.copy(xb_all[:, ic * d:(ic + 1) * d], x_sl)  # f32->bf16
                xs = xsb.tile([128, d], BF16, tag="xs")
                nc.vector.tensor_mul(xs, x_sl, dsc_b)
                for kc in range(2):
                    nc.sync.dma_start_transpose(
                        inputT[:, kc * Li + ic * 128: kc * Li + (ic + 1) * 128],
                        xs[:, kc * 128:(kc + 1) * 128])

            # ---- stage B0: load memory, memory_dot, casts, transposes ----
            for jc in range(NJ):
                mt = mload.tile([128, d], F32, tag="mt")
                nc.sync.dma_start(out=mt, in_=m_d[jc * 128:(jc + 1) * 128, :])
                sc = scr.tile([128, d], F32, tag="scr")
                nc.vector.scalar_tensor_tensor(
                    out=sc, in0=mt, scalar=0.0, in1=wmem_b,
                    op0=ALU.add, op1=ALU.mult, accum_out=mvec[:, jc:jc + 1])
                a0 = jc * (d + 1)
                nc.scalar.copy(maug[:, a0:a0 + d], mt)  # f32->bf16
                nc.vector.memset(maug[:, a0 + d:a0 + d + 1], 1.0)
                for kc in range(2):
                    nc.sync.dma_start_transpose(
                        memT[:, kc * Lmp + jc * 128: kc * Lmp + (jc + 1) * 128],
                        maug[:, a0 + kc * 128: a0 + (kc + 1) * 128])

            # ---- stage B1: bias = mvec + maskpad - (max+4) ----
            nc.vector.tensor_add(mvec, mvec, mp_sb)
            nc.vector.reduce_max(out=cmax, in_=mvec, axis=AX.X)
            ps_c = ps.tile([1, 128], F32, tag="ps")
            nc.tensor.transpose(ps_c, cmax, ident32)
            nc.vector.reduce_max(out=cm1, in_=ps_c, axis=AX.X)
            nc.gpsimd.partition_broadcast(cm_all, cm1)
            nc.vector.tensor_scalar(
                out=bias_sb, in0=mvec, scalar1=cm_all[:, 0:1], scalar2=-4.0,
                op0=ALU.subtract, op1=ALU.add)

            # ---- stage B2: phase-1 matmuls S^T = memT.T @ inputT, exp, max chain ----
            for jc in range(NJ):
                psum_s = ps.tile([128, Li], F32, tag="ps")
                for kc in range(2):
                    for (bs, bn) in banks:
                        nc.tensor.matmul(
                            psum_s[:, bs:bs + bn],
                            memT[:, kc * Lmp + jc * 128: kc * Lmp + (jc + 1) * 128],
                            inputT[:, kc * Li + bs: kc * Li + bs + bn],
                            start=(kc == 0), stop=(kc == 1))
                pt_sl = PT[:, jc * Li:(jc + 1) * Li]
                nc.scalar.activation(out=pt_sl, in_=psum_s, func=ACTF.Exp,
                                     bias=bias_sb[:, jc:jc + 1], scale=1.0)
                if jc == 0:
                    nc.vector.tensor_copy(M1, pt_sl)
                else:
                    nc.vector.tensor_max(M1, M1, pt_sl)

            # ---- stage C: maxP[i] = max over partitions of M1 (PE transpose) ----
            for t in range(NI):
                psT = ps.tile([128, 128], BF16, tag="ps")
                nc.tensor.transpose(psT, M1[:, t * 128:(t + 1) * 128], ident)
                nc.vector.reduce_max(out=maxP[:, t:t + 1], in_=psT, axis=AX.X)

            # ---- stage D: weight_two and output_two ----
            nc.vector.reduce_max(out=k1, in_=idot, axis=AX.X)
            ps_k = ps.tile([1, 128], F32, tag="ps")
            nc.tensor.transpose(ps_k, k1, ident32)
            nc.vector.reduce_max(out=k11, in_=ps_k, axis=AX.X)
            nc.gpsimd.partition_broadcast(k_all, k11)
            nc.vector.tensor_scalar_mul(negk, k_all, -1.0)
            nc.scalar.activation(out=e2, in_=idot, func=ACTF.Exp,
                                 bias=negk[:, 0:1], scale=1.0)
            nc.vector.tensor_mul(u_t, maxP, e2)
            nc.vector.reduce_sum(out=su1, in_=u_t, axis=AX.X)
            ps_u = ps.tile([1, 1], F32, tag="ps")
            nc.tensor.matmul(ps_u, su1, ones32, start=True, stop=True)
            nc.vector.tensor_copy(su11, ps_u)
            nc.gpsimd.partition_broadcast(su_all, su11)
            nc.vector.reciprocal(rec2, su_all)
            nc.vector.tensor_scalar(out=wt2b, in0=u_t, scalar1=rec2[:, 0:1],
                                    scalar2=None, op0=ALU.mult)
            psum_o2 = ps.tile([1, d], F32, tag="ps")
            for ic in range(NI):
                nc.tensor.matmul(psum_o2, wt2b[:, ic:ic + 1],
                                 xb_all[:, ic * d:(ic + 1) * d],
                                 start=(ic == 0), stop=(ic == NI - 1))
            nc.vector.tensor_copy(o2_1, psum_o2)
            nc.gpsimd.partition_broadcast(o2b, o2_1)

            # ---- stage E: phase-2 matmuls O1 = P^T.T @ [memory|1], epilogue ----
            for it in range(NI):
                psum_o = ps.tile([128, d + 1], F32, tag="ps")
                for jc in range(NJ):
                    nc.tensor.matmul(
                        psum_o,
                        PT[:, jc * Li + it * 128: jc * Li + (it + 1) * 128],
                        maug[:, jc * (d + 1):(jc + 1) * (d + 1)],
                        start=(jc == 0), stop=(jc == NJ - 1))
                rec_s = scr.tile([128, 1], F32, tag="rec_s")
                nc.vector.reciprocal(rec_s, psum_o[:, d:d + 1])
                stg = stgp.tile([128, 4 * d], F32, tag="stg")
                x_sl = x_all[:, it * d:(it + 1) * d]
                nc.scalar.copy(stg[:, 0:d], x_sl)
                nc.scalar.mul(stg[:, d:2 * d], psum_o[:, 0:d], rec_s[:, 0:1])
                nc.vector.scalar_tensor_tensor(
                    out=stg[:, 2 * d:3 * d], in0=psum_o[:, 0:d],
                    scalar=rec_s[:, 0:1], in1=x_sl, op0=ALU.mult, op1=ALU.mult)
                nc.vector.scalar_tensor_tensor(
                    out=stg[:, 3 * d:4 * d], in0=psum_o[:, 0:d],
                    scalar=rec_s[:, 0:1], in1=o2b, op0=ALU.mult, op1=ALU.mult)
                nc.sync.dma_start(out=out_d[it * 128:(it + 1) * 128, :], in_=stg)

    nc.compile()
    return nc


def _prep_core_inputs(x_b, m_b, mask_b, w_in, w_mem, dsc, Lmp):
    """Host-side shard prep: permute unmasked memory rows first, pad to Lmp."""
    d = x_b.shape[1]
    idx = np.flatnonzero(mask_b != 0)
    cnt = len(idx)
    m_p = np.zeros((Lmp, d), dtype=np.float32)
    m_p[:cnt] = m_b[idx]
    flat = np.zeros(Lmp, dtype=np.float32)
    flat[cnt:] = -NEG
    mp_t = np.ascontiguousarray(flat.reshape(Lmp // 128, 128).T)  # [128, NJ]
    return {
        "x": np.ascontiguousarray(x_b, dtype=np.float32),
        "m": m_p,
        "mp": mp_t,
        "w_in": np.ascontiguousarray(w_in, dtype=np.float32),
        "w_mem": np.ascontiguousarray(w_mem, dtype=np.float32),
        "dsc": np.ascontiguousarray(dsc, dtype=np.float32),
    }


def kernel(input, memory, mask, w_in, w_mem, dot_scale, _tmpdir=None):
    global LAST_RESULTS
    input = np.asarray(input, dtype=np.float32)
    memory = np.asarray(memory, dtype=np.float32)
    mask = np.asarray(mask)
    w_in = np.asarray(w_in, dtype=np.float32)
    w_mem = np.asarray(w_mem, dtype=np.float32)
    dot_scale = np.asarray(dot_scale, dtype=np.float32)

    bsz, Li, d = input.shape
    assert bsz == N_CORES

    counts = [int((mask[b] != 0).sum()) for b in range(bsz)]
    Lmp = max(128, int(math.ceil(max(counts) / 128.0)) * 128)

    key = (Li, Lmp, d)
    if key not in _NC_CACHE:
        _NC_CACHE[key] = build_nc(Li, Lmp, d)
    nc = _NC_CACHE[key]

    in_maps = [
        _prep_core_inputs(input[b], memory[b], mask[b], w_in, w_mem, dot_scale, Lmp)
        for b in range(bsz)
    ]
    res = run_bass_kernel_spmd(nc, in_maps, list(range(N_CORES)), tmpdir=_tmpdir)
    LAST_RESULTS = res
    out = np.stack([res.results[b]["out"] for b in range(bsz)], axis=0)
    return out


# revision 15
# speedup vs baseline: 1.5322x; 1.5322x over previous
"""BiAttention Trainium2 Bass kernel.

Per-core (one batch per NeuronCore, batch=8 over 8 cores):
  att[i,j] = input_dot[i] + memory_dot[j] + (input*dot_scale) @ memory^T - NEG*(1-mask[j])
  weight_one = softmax_j(att);  output_one = weight_one @ memory
  weight_two = softmax_i(max_j att);  output_two = weight_two @ input
  out = concat([input, output_one, input*output_one, output_two*output_one], -1)

Implementation notes:
  - input_dot[i] is constant along j, so it cancels in softmax_j; only
    memory_dot + mask enter the attention bias (per-j "mvec").
  - Rows of `memory` (and the additive mask) are permuted host-side so that
    unmasked rows come first; masked rows never reach the device (sum over j is
    permutation invariant).  Only Lmp = ceil(count/128)*128 rows are computed.
  - Scores are built transposed (S^T[j,i]) so mvec is a per-partition ACT bias
    and exp(S^T + mvec - C) lands directly in the P^T layout that the second
    matmul (contraction over j) needs.  C = max(mvec)+4 is a safe global shift.
  - max_j att (needed for weight_two) is recovered as C + log(max_j expvals);
    the log never materializes: weight_two numerator uses maxP * exp(input_dot-K).
  - denominator sum_j comes for free from an appended ones-column in memory.
"""

import math
import numpy as np

import concourse.bass as bass
import concourse.mybir as mybir
import concourse.tile as tile
import concourse.bacc as bacc
from concourse import bass_isa
from concourse.bass_utils import run_bass_kernel_spmd
from concourse.masks import make_identity

F32 = mybir.dt.float32
BF16 = mybir.dt.bfloat16
AX = mybir.AxisListType
ALU = mybir.AluOpType
ACTF = mybir.ActivationFunctionType

N_CORES = 8
NEG = 1e30

_NC_CACHE: dict = {}
LAST_RESULTS = None  # BassKernelResults of the most recent run (for test harness)


def build_nc(Li: int, Lmp: int, d: int):
    """Build the single-core SPMD program.  Li, d fixed; Lmp = padded #unmasked."""
    assert Li % 128 == 0 and Lmp % 128 == 0 and d == 256
    NI = Li // 128
    NJ = Lmp // 128
    banks = [(s, min(512, Li - s)) for s in range(0, Li, 512)]

    nc = bacc.Bacc("TRN2", target_bir_lowering=False, debug=False,
                   num_devices=N_CORES)

    x_d = nc.dram_tensor("x", [Li, d], F32, kind="ExternalInput")
    m_d = nc.dram_tensor("m", [Lmp, d], F32, kind="ExternalInput")
    mp_d = nc.dram_tensor("mp", [128, NJ], F32, kind="ExternalInput")
    win_d = nc.dram_tensor("w_in", [d], F32, kind="ExternalInput")
    wmem_d = nc.dram_tensor("w_mem", [d], F32, kind="ExternalInput")
    dsc_d = nc.dram_tensor("dsc", [d], F32, kind="ExternalInput")
    out_d = nc.dram_tensor("out", [Li, 4 * d], F32, kind="ExternalOutput")

    with tile.TileContext(nc) as tc:
        with (
            tc.tile_pool(name="singles", bufs=1) as singles,
            tc.tile_pool(name="mload", bufs=3) as mload,
            tc.tile_pool(name="xsb", bufs=3) as xsb,
            tc.tile_pool(name="scr", bufs=3) as scr,
            tc.tile_pool(name="stg", bufs=3) as stgp,
            tc.tile_pool(name="ps", bufs=2, space="PSUM") as ps,
            tc.tile_pool(name="dram", bufs=1, space="DRAM") as dramp,
        ):
            # ---- constants / broadcasts ----
            win_b = singles.tile([128, d], F32, tag="win_b")
            wmem_b = singles.tile([128, d], F32, tag="wmem_b")
            dsc_b = singles.tile([128, d], F32, tag="dsc_b")
            nc.sync.dma_start(out=win_b, in_=win_d.ap().unsqueeze(0).partition_broadcast(128))
            nc.sync.dma_start(out=wmem_b, in_=wmem_d.ap().unsqueeze(0).partition_broadcast(128))
            nc.sync.dma_start(out=dsc_b, in_=dsc_d.ap().unsqueeze(0).partition_broadcast(128))

            ident = singles.tile([128, 128], BF16, tag="ident")
            make_identity(nc, ident)
            ident32 = singles.tile([128, 128], F32, tag="ident32")
            make_identity(nc, ident32)
            ones32 = singles.tile([128, 1], F32, tag="ones32")
            nc.vector.memset(ones32, 1.0)

            mp_sb = singles.tile([128, NJ], F32, tag="mp_sb")
            nc.sync.dma_start(out=mp_sb, in_=mp_d[:, :])

            # ---- resident big tiles ----
            x_all = singles.tile([128, NI * d], F32, tag="x_all")
            xb_all = singles.tile([128, NI * d], BF16, tag="xb_all")
            inputT = singles.tile([128, 2 * Li], BF16, tag="inputT")  # [d-half, i]
            memT = singles.tile([128, 2 * Lmp], BF16, tag="memT")     # [d-half, j]
            maug = singles.tile([128, NJ * (d + 1)], BF16, tag="maug")
            PT = singles.tile([128, NJ * Li], BF16, tag="PT")         # exp scores^T
            M1 = singles.tile([128, Li], BF16, tag="M1")              # running max of PT

            # ---- small stats ----
            idot = singles.tile([128, NI], F32, tag="idot")
            mvec = singles.tile([128, NJ], F32, tag="mvec")
            bias_sb = singles.tile([128, NJ], F32, tag="bias_sb")
            maxP = singles.tile([128, NI], F32, tag="maxP")
            cmax = singles.tile([128, 1], F32, tag="cmax")
            cm1 = singles.tile([1, 1], F32, tag="cm1")
            cm_all = singles.tile([128, 1], F32, tag="cm_all")
            k1 = singles.tile([128, 1], F32, tag="k1")
            k11 = singles.tile([1, 1], F32, tag="k11")
            k_all = singles.tile([128, 1], F32, tag="k_all")
            negk = singles.tile([128, 1], F32, tag="negk")
            e2 = singles.tile([128, NI], F32, tag="e2")
            u_t = singles.tile([128, NI], F32, tag="u_t")
            su1 = singles.tile([128, 1], F32, tag="su1")
            su11 = singles.tile([1, 1], F32, tag="su11")
            su_all = singles.tile([128, 1], F32, tag="su_all")
            rec2 = singles.tile([128, 1], F32, tag="rec2")
            wt2b = singles.tile([128, NI], BF16, tag="wt2b")
            o2_1 = singles.tile([1, d], F32, tag="o2_1")
            o2b = singles.tile([128, d], F32, tag="o2b")

            xsd = dramp.tile([Li, d], BF16, tag="xsd")
            msd = dramp.tile([Lmp, d], BF16, tag="msd")

            # ---- stage A: load input, input_dot, casts, transposes ----
            for ic in range(NI):
                x_sl = x_all[:, ic * d:(ic + 1) * d]
                nc.sync.dma_start(out=x_sl, in_=x_d[ic * 128:(ic + 1) * 128, :])
                sc = scr.tile([128, d], F32, tag="scr")
                nc.vector.scalar_tensor_tensor(
                    out=sc, in0=x_sl, scalar=0.0, in1=win_b,
                    op0=ALU.add, op1=ALU.mult, accum_out=idot[:, ic:ic + 1])
                nc.scalar.copy(xb_all[:, ic * d:(ic + 1) * d], x_sl)  # f32->bf16
                xs = xsb.tile([128, d], BF16, tag="xs")
                nc.vector.tensor_mul(xs, x_sl, dsc_b)
                nc.sync.dma_start(out=xsd[ic * 128:(ic + 1) * 128, :], in_=xs)
            # one big DRAM->SBUF transpose-load per d-half
            for kc in range(2):
                nc.sync.dma_start(out=inputT[:, kc * Li:(kc + 1) * Li],
                                  in_=xsd[:, kc * 128:(kc + 1) * 128],
                                  transpose=True)

            # ---- stage B0: load memory, memory_dot, casts, transposes ----
            for jc in range(NJ):
                mt = mload.tile([128, d], F32, tag="mt")
                nc.sync.dma_start(out=mt, in_=m_d[jc * 128:(jc + 1) * 128, :])
                sc = scr.tile([128, d], F32, tag="scr")
                nc.vector.scalar_tensor_tensor(
                    out=sc, in0=mt, scalar=0.0, in1=wmem_b,
                    op0=ALU.add, op1=ALU.mult, accum_out=mvec[:, jc:jc + 1])
                a0 = jc * (d + 1)
                nc.scalar.copy(maug[:, a0:a0 + d], mt)  # f32->bf16
                nc.vector.memset(maug[:, a0 + d:a0 + d + 1], 1.0)
                nc.sync.dma_start(out=msd[jc * 128:(jc + 1) * 128, :],
                                  in_=maug[:, a0:a0 + d])
            for kc in range(2):
                nc.sync.dma_start(out=memT[:, kc * Lmp:(kc + 1) * Lmp],
                                  in_=msd[:, kc * 128:(kc + 1) * 128],
                                  transpose=True)

            # ---- stage B1: bias = mvec + maskpad - (max+4) ----
            nc.vector.tensor_add(mvec, mvec, mp_sb)
            nc.vector.reduce_max(out=cmax, in_=mvec, axis=AX.X)
            ps_c = ps.tile([1, 128], F32, tag="ps")
            nc.tensor.transpose(ps_c, cmax, ident32)
            nc.vector.reduce_max(out=cm1, in_=ps_c, axis=AX.X)
            nc.gpsimd.partition_broadcast(cm_all, cm1)
            nc.vector.tensor_scalar(
                out=bias_sb, in0=mvec, scalar1=cm_all[:, 0:1], scalar2=-4.0,
                op0=ALU.subtract, op1=ALU.add)

            # ---- stage B2: phase-1 matmuls S^T = memT.T @ inputT, exp, max chain ----
            for jc in range(NJ):
                psum_s = ps.tile([128, Li], F32, tag="ps")
                for kc in range(2):
                    for (bs, bn) in banks:
                        nc.tensor.matmul(
                            psum_s[:, bs:bs + bn],
                            memT[:, kc * Lmp + jc * 128: kc * Lmp + (jc + 1) * 128],
                            inputT[:, kc * Li + bs: kc * Li + bs + bn],
                            start=(kc == 0), stop=(kc == 1))
                pt_sl = PT[:, jc * Li:(jc + 1) * Li]
                nc.scalar.activation(out=pt_sl, in_=psum_s, func=ACTF.Exp,
                                     bias=bias_sb[:, jc:jc + 1], scale=1.0)
                if jc == 0:
                    nc.vector.tensor_copy(M1, pt_sl)
                else:
                    nc.vector.tensor_max(M1, M1, pt_sl)

            # ---- stage C: maxP[i] = max over partitions of M1 (PE transpose) ----
            for t in range(NI):
                psT = ps.tile([128, 128], BF16, tag="ps")
                nc.tensor.transpose(psT, M1[:, t * 128:(t + 1) * 128], ident)
                nc.vector.reduce_max(out=maxP[:, t:t + 1], in_=psT, axis=AX.X)

            # ---- stage D: weight_two and output_two ----
            nc.vector.reduce_max(out=k1, in_=idot, axis=AX.X)
            ps_k = ps.tile([1, 128], F32, tag="ps")
            nc.tensor.transpose(ps_k, k1, ident32)
            nc.vector.reduce_max(out=k11, in_=ps_k, axis=AX.X)
            nc.gpsimd.partition_broadcast(k_all, k11)
            nc.vector.tensor_scalar_mul(negk, k_all, -1.0)
            nc.scalar.activation(out=e2, in_=idot, func=ACTF.Exp,
                                 bias=negk[:, 0:1], scale=1.0)
            nc.vector.tensor_mul(u_t, maxP, e2)
            nc.vector.reduce_sum(out=su1, in_=u_t, axis=AX.X)
            ps_u = ps.tile([1, 1], F32, tag="ps")
            nc.tensor.matmul(ps_u, su1, ones32, start=True, stop=True)
            nc.vector.tensor_copy(su11, ps_u)
            nc.gpsimd.partition_broadcast(su_all, su11)
            nc.vector.reciprocal(rec2, su_all)
            nc.vector.tensor_scalar(out=wt2b, in0=u_t, scalar1=rec2[:, 0:1],
                                    scalar2=None, op0=ALU.mult)
            psum_o2 = ps.tile([1, d], F32, tag="ps")
            for ic in range(NI):
                nc.tensor.matmul(psum_o2, wt2b[:, ic:ic + 1],
                                 xb_all[:, ic * d:(ic + 1) * d],
                                 start=(ic == 0), stop=(ic == NI - 1))
            nc.vector.tensor_copy(o2_1, psum_o2)
            nc.gpsimd.partition_broadcast(o2b, o2_1)

            # ---- stage E: phase-2 matmuls O1 = P^T.T @ [memory|1], epilogue ----
            for it in range(NI):
                psum_o = ps.tile([128, d + 1], F32, tag="ps")
                for jc in range(NJ):
                    nc.tensor.matmul(
                        psum_o,
                        PT[:, jc * Li + it * 128: jc * Li + (it + 1) * 128],
                        maug[:, jc * (d + 1):(jc + 1) * (d + 1)],
                        start=(jc == 0), stop=(jc == NJ - 1))
                rec_s = scr.tile([128, 1], F32, tag="rec_s")
                nc.vector.reciprocal(rec_s, psum_o[:, d:d + 1])
                stg = stgp.tile([128, 4 * d], F32, tag="stg")
                x_sl = x_all[:, it * d:(it + 1) * d]
                nc.scalar.copy(stg[:, 0:d], x_sl)
                nc.scalar.mul(stg[:, d:2 * d], psum_o[:, 0:d], rec_s[:, 0:1])
                nc.vector.scalar_tensor_tensor(
                    out=stg[:, 2 * d:3 * d], in0=psum_o[:, 0:d],
                    scalar=rec_s[:, 0:1], in1=x_sl, op0=ALU.mult, op1=ALU.mult)
                nc.vector.scalar_tensor_tensor(
                    out=stg[:, 3 * d:4 * d], in0=psum_o[:, 0:d],
                    scalar=rec_s[:, 0:1], in1=o2b, op0=ALU.mult, op1=ALU.mult)
                nc.sync.dma_start(out=out_d[it * 128:(it + 1) * 128, :], in_=stg)

    nc.compile()
    return nc


def _prep_core_inputs(x_b, m_b, mask_b, w_in, w_mem, dsc, Lmp):
    """Host-side shard prep: permute unmasked memory rows first, pad to Lmp."""
    d = x_b.shape[1]
    idx = np.flatnonzero(mask_b != 0)
    cnt = len(idx)
    m_p = np.zeros((Lmp, d), dtype=np.float32)
    m_p[:cnt] = m_b[idx]
    flat = np.zeros(Lmp, dtype=np.float32)
    flat[cnt:] = -NEG
    mp_t = np.ascontiguousarray(flat.reshape(Lmp // 128, 128).T)  # [128, NJ]
    return {
        "x": np.ascontiguousarray(x_b, dtype=np.float32),
        "m": m_p,
        "mp": mp_t,
        "w_in": np.ascontiguousarray(w_in, dtype=np.float32),
        "w_mem": np.ascontiguousarray(w_mem, dtype=np.float32),
        "dsc": np.ascontiguousarray(dsc, dtype=np.float32),
    }


def kernel(input, memory, mask, w_in, w_mem, dot_scale, _tmpdir=None):
    global LAST_RESULTS
    input = np.asarray(input, dtype=np.float32)
    memory = np.asarray(memory, dtype=np.float32)
    mask = np.asarray(mask)
    w_in = np.asarray(w_in, dtype=np.float32)
    w_mem = np.asarray(w_mem, dtype=np.float32)
    dot_scale = np.asarray(dot_scale, dtype=np.float32)

    bsz, Li, d = input.shape
    assert bsz == N_CORES

    counts = [int((mask[b] != 0).sum()) for b in range(bsz)]
    Lmp = max(128, int(math.ceil(max(counts) / 128.0)) * 128)

    key = (Li, Lmp, d)
    if key not in _NC_CACHE:
        _NC_CACHE[key] = build_nc(Li, Lmp, d)
    nc = _NC_CACHE[key]

    in_maps = [
        _prep_core_inputs(input[b], memory[b], mask[b], w_in, w_mem, dot_scale, Lmp)
        for b in range(bsz)
    ]
    res = run_bass_kernel_spmd(nc, in_maps, list(range(N_CORES)), tmpdir=_tmpdir)
    LAST_RESULTS = res
    out = np.stack([res.results[b]["out"] for b in range(bsz)], axis=0)
    return out


# revision 16
# speedup vs baseline: 1.7917x; 1.1694x over previous
"""BiAttention Trainium2 Bass kernel.

Per-core (one batch per NeuronCore, batch=8 over 8 cores):
  att[i,j] = input_dot[i] + memory_dot[j] + (input*dot_scale) @ memory^T - NEG*(1-mask[j])
  weight_one = softmax_j(att);  output_one = weight_one @ memory
  weight_two = softmax_i(max_j att);  output_two = weight_two @ input
  out = concat([input, output_one, input*output_one, output_two*output_one], -1)

Implementation notes:
  - input_dot[i] is constant along j, so it cancels in softmax_j; only
    memory_dot + mask enter the attention bias (per-j "mvec").
  - Rows of `memory` (and the additive mask) are permuted host-side so that
    unmasked rows come first; masked rows never reach the device (sum over j is
    permutation invariant).  Only Lmp = ceil(count/128)*128 rows are computed.
  - Scores are built transposed (S^T[j,i]) so mvec is a per-partition ACT bias
    and exp(S^T + mvec - C) lands directly in the P^T layout that the second
    matmul (contraction over j) needs.  C = max(mvec)+4 is a safe global shift.
  - max_j att (needed for weight_two) is recovered as C + log(max_j expvals);
    the log never materializes: weight_two numerator uses maxP * exp(input_dot-K).
  - denominator sum_j comes for free from an appended ones-column in memory.
"""

import math
import numpy as np

import concourse.bass as bass
import concourse.mybir as mybir
import concourse.tile as tile
import concourse.bacc as bacc
from concourse import bass_isa
from concourse.bass_utils import run_bass_kernel_spmd
from concourse.masks import make_identity

F32 = mybir.dt.float32
BF16 = mybir.dt.bfloat16
AX = mybir.AxisListType
ALU = mybir.AluOpType
ACTF = mybir.ActivationFunctionType

N_CORES = 8
NEG = 1e30

_NC_CACHE: dict = {}
LAST_RESULTS = None  # BassKernelResults of the most recent run (for test harness)


def build_nc(Li: int, Lmp: int, d: int):
    """Build the single-core SPMD program.  Li, d fixed; Lmp = padded #unmasked."""
    assert Li % 128 == 0 and Lmp % 128 == 0 and d == 256
    NI = Li // 128
    NJ = Lmp // 128
    banks = [(s, min(512, Li - s)) for s in range(0, Li, 512)]

    nc = bacc.Bacc("TRN2", target_bir_lowering=False, debug=False,
                   num_devices=N_CORES)

    x_d = nc.dram_tensor("x", [Li, d], F32, kind="ExternalInput")
    m_d = nc.dram_tensor("m", [Lmp, d], F32, kind="ExternalInput")
    mp_d = nc.dram_tensor("mp", [128, NJ], F32, kind="ExternalInput")
    win_d = nc.dram_tensor("w_in", [d], F32, kind="ExternalInput")
    wmem_d = nc.dram_tensor("w_mem", [d], F32, kind="ExternalInput")
    dsc_d = nc.dram_tensor("dsc", [d], F32, kind="ExternalInput")
    out_d = nc.dram_tensor("out", [Li, 4 * d], F32, kind="ExternalOutput")

    with tile.TileContext(nc) as tc:
        with (
            tc.tile_pool(name="singles", bufs=1) as singles,
            tc.tile_pool(name="mload", bufs=3) as mload,
            tc.tile_pool(name="xsb", bufs=3) as xsb,
            tc.tile_pool(name="scr", bufs=3) as scr,
            tc.tile_pool(name="stg", bufs=3) as stgp,
            tc.tile_pool(name="ps", bufs=2, space="PSUM") as ps,
            tc.tile_pool(name="dram", bufs=1, space="DRAM") as dramp,
        ):
            # ---- constants / broadcasts ----
            win_b = singles.tile([128, d], F32, tag="win_b")
            wmem_b = singles.tile([128, d], F32, tag="wmem_b")
            dsc_b = singles.tile([128, d], F32, tag="dsc_b")
            nc.sync.dma_start(out=win_b, in_=win_d.ap().unsqueeze(0).partition_broadcast(128))
            nc.sync.dma_start(out=wmem_b, in_=wmem_d.ap().unsqueeze(0).partition_broadcast(128))
            nc.sync.dma_start(out=dsc_b, in_=dsc_d.ap().unsqueeze(0).partition_broadcast(128))

            ident = singles.tile([128, 128], BF16, tag="ident")
            make_identity(nc, ident)
            ident32 = singles.tile([128, 128], F32, tag="ident32")
            make_identity(nc, ident32)
            ones32 = singles.tile([128, 1], F32, tag="ones32")
            nc.vector.memset(ones32, 1.0)

            mp_sb = singles.tile([128, NJ], F32, tag="mp_sb")
            nc.sync.dma_start(out=mp_sb, in_=mp_d[:, :])

            # ---- resident big tiles ----
            x_all = singles.tile([128, NI * d], F32, tag="x_all")
            xb_all = singles.tile([128, NI * d], BF16, tag="xb_all")
            inputT = singles.tile([128, 2 * Li], BF16, tag="inputT")  # [d-half, i]
            memT = singles.tile([128, 2 * Lmp], BF16, tag="memT")     # [d-half, j]
            maug = singles.tile([128, NJ * (d + 1)], BF16, tag="maug")
            PT = singles.tile([128, NJ * Li], BF16, tag="PT")         # exp scores^T
            M1 = singles.tile([128, Li], BF16, tag="M1")              # running max of PT

            # ---- small stats ----
            idot = singles.tile([128, NI], F32, tag="idot")
            mvec = singles.tile([128, NJ], F32, tag="mvec")
            bias_sb = singles.tile([128, NJ], F32, tag="bias_sb")
            maxP = singles.tile([128, NI], F32, tag="maxP")
            cmax = singles.tile([128, 1], F32, tag="cmax")
            cm1 = singles.tile([1, 1], F32, tag="cm1")
            cm_all = singles.tile([128, 1], F32, tag="cm_all")
            k1 = singles.tile([128, 1], F32, tag="k1")
            k11 = singles.tile([1, 1], F32, tag="k11")
            k_all = singles.tile([128, 1], F32, tag="k_all")
            negk = singles.tile([128, 1], F32, tag="negk")
            e2 = singles.tile([128, NI], F32, tag="e2")
            u_t = singles.tile([128, NI], F32, tag="u_t")
            su1 = singles.tile([128, 1], F32, tag="su1")
            su11 = singles.tile([1, 1], F32, tag="su11")
            su_all = singles.tile([128, 1], F32, tag="su_all")
            rec2 = singles.tile([128, 1], F32, tag="rec2")
            wt2b = singles.tile([128, NI], BF16, tag="wt2b")
            o2_1 = singles.tile([1, d], F32, tag="o2_1")
            o2b = singles.tile([128, d], F32, tag="o2b")

            xsd = dramp.tile([Li, d], BF16, tag="xsd")
            msd = dramp.tile([Lmp, d], BF16, tag="msd")
            xs_all = singles.tile([128, NI * d], BF16, tag="xs_all")
            m_all = singles.tile([128, NJ * d], F32, tag="m_all")

            # ---- stage A: load input, input_dot, casts, transposes ----
            QI = max(1, NI // 4)  # chunks per load/store group
            for g in range(0, NI, QI):
                ge = min(g + QI, NI)
                nc.sync.dma_start(
                    out=x_all[:, g * d:ge * d].rearrange("p (c x) -> p c x", x=d),
                    in_=x_d[g * 128:ge * 128, :].rearrange("(c p) x -> p c x", p=128))
            for ic in range(NI):
                x_sl = x_all[:, ic * d:(ic + 1) * d]
                sc = scr.tile([128, d], F32, tag="scr")
                nc.vector.scalar_tensor_tensor(
                    out=sc, in0=x_sl, scalar=0.0, in1=win_b,
                    op0=ALU.add, op1=ALU.mult, accum_out=idot[:, ic:ic + 1])
                nc.scalar.copy(xb_all[:, ic * d:(ic + 1) * d], x_sl)  # f32->bf16
                nc.vector.tensor_mul(xs_all[:, ic * d:(ic + 1) * d], x_sl, dsc_b)
            for g in range(0, NI, QI):
                ge = min(g + QI, NI)
                nc.sync.dma_start(
                    out=xsd[g * 128:ge * 128, :].rearrange("(c p) x -> p c x", p=128),
                    in_=xs_all[:, g * d:ge * d].rearrange("p (c x) -> p c x", x=d))
            # one big DRAM->SBUF transpose-load per d-half
            for kc in range(2):
                nc.sync.dma_start(out=inputT[:, kc * Li:(kc + 1) * Li],
                                  in_=xsd[:, kc * 128:(kc + 1) * 128],
                                  transpose=True)

            # ---- stage B0: load memory, memory_dot, casts, transposes ----
            QJ = max(1, (NJ + 1) // 2)
            for g in range(0, NJ, QJ):
                ge = min(g + QJ, NJ)
                nc.sync.dma_start(
                    out=m_all[:, g * d:ge * d].rearrange("p (c x) -> p c x", x=d),
                    in_=m_d[g * 128:ge * 128, :].rearrange("(c p) x -> p c x", p=128))
            maug_r = maug[:].rearrange("p (c x) -> p c x", x=d + 1)
            for jc in range(NJ):
                mt = m_all[:, jc * d:(jc + 1) * d]
                sc = scr.tile([128, d], F32, tag="scr")
                nc.vector.scalar_tensor_tensor(
                    out=sc, in0=mt, scalar=0.0, in1=wmem_b,
                    op0=ALU.add, op1=ALU.mult, accum_out=mvec[:, jc:jc + 1])
                a0 = jc * (d + 1)
                nc.scalar.copy(maug[:, a0:a0 + d], mt)  # f32->bf16
                nc.vector.memset(maug[:, a0 + d:a0 + d + 1], 1.0)
            for g in range(0, NJ, QJ):
                ge = min(g + QJ, NJ)
                nc.sync.dma_start(
                    out=msd[g * 128:ge * 128, :].rearrange("(c p) x -> p c x", p=128),
                    in_=maug_r[:, g:ge, 0:d])
            for kc in range(2):
                nc.sync.dma_start(out=memT[:, kc * Lmp:(kc + 1) * Lmp],
                                  in_=msd[:, kc * 128:(kc + 1) * 128],
                                  transpose=True)

            # ---- stage B1: bias = mvec + maskpad - (max+4) ----
            nc.vector.tensor_add(mvec, mvec, mp_sb)
            nc.vector.reduce_max(out=cmax, in_=mvec, axis=AX.X)
            ps_c = ps.tile([1, 128], F32, tag="ps")
            nc.tensor.transpose(ps_c, cmax, ident32)
            nc.vector.reduce_max(out=cm1, in_=ps_c, axis=AX.X)
            nc.gpsimd.partition_broadcast(cm_all, cm1)
            nc.vector.tensor_scalar(
                out=bias_sb, in0=mvec, scalar1=cm_all[:, 0:1], scalar2=-4.0,
                op0=ALU.subtract, op1=ALU.add)

            # ---- stage B2: phase-1 matmuls S^T = memT.T @ inputT, exp, max chain ----
            for jc in range(NJ):
                psum_s = ps.tile([128, Li], F32, tag="ps")
                for kc in range(2):
                    for (bs, bn) in banks:
                        nc.tensor.matmul(
                            psum_s[:, bs:bs + bn],
                            memT[:, kc * Lmp + jc * 128: kc * Lmp + (jc + 1) * 128],
                            inputT[:, kc * Li + bs: kc * Li + bs + bn],
                            start=(kc == 0), stop=(kc == 1))
                pt_sl = PT[:, jc * Li:(jc + 1) * Li]
                nc.scalar.activation(out=pt_sl, in_=psum_s, func=ACTF.Exp,
                                     bias=bias_sb[:, jc:jc + 1], scale=1.0)
                if jc == 0:
                    nc.vector.tensor_copy(M1, pt_sl)
                else:
                    nc.vector.tensor_max(M1, M1, pt_sl)

            # ---- stage C: maxP[i] = max over partitions of M1 (PE transpose) ----
            for t in range(NI):
                psT = ps.tile([128, 128], BF16, tag="ps")
                nc.tensor.transpose(psT, M1[:, t * 128:(t + 1) * 128], ident)
                nc.vector.reduce_max(out=maxP[:, t:t + 1], in_=psT, axis=AX.X)

            # ---- stage D: weight_two and output_two ----
            nc.vector.reduce_max(out=k1, in_=idot, axis=AX.X)
            ps_k = ps.tile([1, 128], F32, tag="ps")
            nc.tensor.transpose(ps_k, k1, ident32)
            nc.vector.reduce_max(out=k11, in_=ps_k, axis=AX.X)
            nc.gpsimd.partition_broadcast(k_all, k11)
            nc.vector.tensor_scalar_mul(negk, k_all, -1.0)
            nc.scalar.activation(out=e2, in_=idot, func=ACTF.Exp,
                                 bias=negk[:, 0:1], scale=1.0)
            nc.vector.tensor_mul(u_t, maxP, e2)
            nc.vector.reduce_sum(out=su1, in_=u_t, axis=AX.X)
            ps_u = ps.tile([1, 1], F32, tag="ps")
            nc.tensor.matmul(ps_u, su1, ones32, start=True, stop=True)
            nc.vector.tensor_copy(su11, ps_u)
            nc.gpsimd.partition_broadcast(su_all, su11)
            nc.vector.reciprocal(rec2, su_all)
            nc.vector.tensor_scalar(out=wt2b, in0=u_t, scalar1=rec2[:, 0:1],
                                    scalar2=None, op0=ALU.mult)
            psum_o2 = ps.tile([1, d], F32, tag="ps")
            for ic in range(NI):
                nc.tensor.matmul(psum_o2, wt2b[:, ic:ic + 1],
                                 xb_all[:, ic * d:(ic + 1) * d],
                                 start=(ic == 0), stop=(ic == NI - 1))
            nc.vector.tensor_copy(o2_1, psum_o2)
            nc.gpsimd.partition_broadcast(o2b, o2_1)

            # ---- stage E: phase-2 matmuls O1 = P^T.T @ [memory|1], epilogue ----
            for it in range(NI):
                psum_o = ps.tile([128, d + 1], F32, tag="ps")
                for jc in range(NJ):
                    nc.tensor.matmul(
                        psum_o,
                        PT[:, jc * Li + it * 128: jc * Li + (it + 1) * 128],
                        maug[:, jc * (d + 1):(jc + 1) * (d + 1)],
                        start=(jc == 0), stop=(jc == NJ - 1))
                rec_s = scr.tile([128, 1], F32, tag="rec_s")
                nc.vector.reciprocal(rec_s, psum_o[:, d:d + 1])
                stg = stgp.tile([128, 4 * d], F32, tag="stg")
                x_sl = x_all[:, it * d:(it + 1) * d]
                nc.scalar.copy(stg[:, 0:d], x_sl)
                nc.scalar.mul(stg[:, d:2 * d], psum_o[:, 0:d], rec_s[:, 0:1])
                nc.vector.scalar_tensor_tensor(
                    out=stg[:, 2 * d:3 * d], in0=psum_o[:, 0:d],
                    scalar=rec_s[:, 0:1], in1=x_sl, op0=ALU.mult, op1=ALU.mult)
                nc.vector.scalar_tensor_tensor(
                    out=stg[:, 3 * d:4 * d], in0=psum_o[:, 0:d],
                    scalar=rec_s[:, 0:1], in1=o2b, op0=ALU.mult, op1=ALU.mult)
                eng = nc.sync if it % 2 == 0 else nc.scalar
                eng.dma_start(out=out_d[it * 128:(it + 1) * 128, :], in_=stg)

    nc.compile()
    return nc


def _prep_core_inputs(x_b, m_b, mask_b, w_in, w_mem, dsc, Lmp):
    """Host-side shard prep: permute unmasked memory rows first, pad to Lmp."""
    d = x_b.shape[1]
    idx = np.flatnonzero(mask_b != 0)
    cnt = len(idx)
    m_p = np.zeros((Lmp, d), dtype=np.float32)
    m_p[:cnt] = m_b[idx]
    flat = np.zeros(Lmp, dtype=np.float32)
    flat[cnt:] = -NEG
    mp_t = np.ascontiguousarray(flat.reshape(Lmp // 128, 128).T)  # [128, NJ]
    return {
        "x": np.ascontiguousarray(x_b, dtype=np.float32),
        "m": m_p,
        "mp": mp_t,
        "w_in": np.ascontiguousarray(w_in, dtype=np.float32),
        "w_mem": np.ascontiguousarray(w_mem, dtype=np.float32),
        "dsc": np.ascontiguousarray(dsc, dtype=np.float32),
    }


def kernel(input, memory, mask, w_in, w_mem, dot_scale, _tmpdir=None):
    global LAST_RESULTS
    input = np.asarray(input, dtype=np.float32)
    memory = np.asarray(memory, dtype=np.float32)
    mask = np.asarray(mask)
    w_in = np.asarray(w_in, dtype=np.float32)
    w_mem = np.asarray(w_mem, dtype=np.float32)
    dot_scale = np.asarray(dot_scale, dtype=np.float32)

    bsz, Li, d = input.shape
    assert bsz == N_CORES

    counts = [int((mask[b] != 0).sum()) for b in range(bsz)]
    Lmp = max(128, int(math.ceil(max(counts) / 128.0)) * 128)

    key = (Li, Lmp, d)
    if key not in _NC_CACHE:
        _NC_CACHE[key] = build_nc(Li, Lmp, d)
    nc = _NC_CACHE[key]

    in_maps = [
        _prep_core_inputs(input[b], memory[b], mask[b], w_in, w_mem, dot_scale, Lmp)
        for b in range(bsz)
    ]
    res = run_bass_kernel_spmd(nc, in_maps, list(range(N_CORES)), tmpdir=_tmpdir)
    LAST_RESULTS = res
    out = np.stack([res.results[b]["out"] for b in range(bsz)], axis=0)
    return out
